# revision 34
# baseline (speedup 1.0000x reference)
"""GQA kernel for 8 Trainium2 NeuronCores.

Problem: nn_GroupQueryAttention — B=2, S=2048, HIDDEN=2048, 32 heads,
8 kv-groups, head_dim 64.

Sharding: data parallel on batch (2) x tensor parallel on kv-groups (4
group-pairs). Core c owns batch c//4 and kv-groups {2*(c%4), 2*(c%4)+1}
(512 q-features, 128 kv-features). Each core computes a partial
out-projection (Wo columns of its features); host sums 4 partials per
batch.

Key optimizations over the fp32r baseline:
  - all matmul operands in bf16 (fp32r ran under a HW power throttle at
    ~1.2GHz; bf16 streams 1 col/cycle at full clock). PSUM accumulation
    stays fp32.
  - key compaction: the mask is per-key (broadcast over queries+heads),
    so masked keys contribute nothing anywhere. Host gathers the
    unmasked key positions, pads to a multiple of 128, and the kernel
    only projects/attends over the kept keys (~half of 2048 for the
    random mask). Padded key rows get an exp bias of -30000 so E=0.
  - scores matmuls are 64-deep (head_dim): the two q-parities of a head
    column live on disjoint SBUF partition halves (kta=[g0;g1],
    ktb=[g1;g0]), so their matmuls run CONCURRENTLY as 64x128 row tiles
    (tile_position (0,0) / (64,0)) — 2x scores throughput. Scores for
    two key blocks are batched per row-mode burst to halve the 64<->128
    row-mode switch drains.
  - one exp + one normalize chain per (head-column, key-block):
    [128, 1024] activation covering both parities (same key-block bias).
  - k/v projections run first behind the small wk/wv DMAs (wk + first
    xk chunk split fine and issued from different engines so the PE
    starts early); wq/wo stream during kv-proj.
  - q-proj of tile qt+1 and out-projection of tile qt-1 are interleaved
    into tile qt's attention round boundaries to fill normalize bubbles.
    qproj PSUM->SBUF copies go on the vector engine (scalar is
    exp-saturated during attention).
  - softmax division: PV's stationary columns 64:128 are all ones, so
    the PE replicates the denominator across 64 PSUM partitions for
    free; the normalize chain is copy -> approx-reciprocal -> multiply,
    all on DVE (no gpsimd partition_broadcast).
  - bf16 output partials summed on host in fp32.
"""

import numpy as np

B = 2
S = 2048
H = 2048
G = 8            # kv groups total
HPG = 4          # heads per group
D = 64           # head dim
NCORES = 8
QF = 512         # q features per core (2 groups * 4 heads * 64)
KF = 128         # kv features per core (2 groups * 64)
SCALE = 1.0 / np.sqrt(np.float32(D))
P = 128
SQA = 512        # seq chunk for projection phase (moving dim)
SQB = 512        # q tile for attention / out-proj phase
NHT = H // P     # 16 hidden partition tiles
NMT = QF // P    # 4 q-feature partition tiles
NQT = S // SQB   # 4 q tiles
MASK_NEG = -30000.0


def _build_bass(KB):
    """Build the per-core program for KPAD = KB*128 kept+padded keys."""
    from contextlib import ExitStack

    import concourse.tile as tile
    from concourse import bacc, mybir

    f32 = mybir.dt.float32
    bf16 = mybir.dt.bfloat16
    Exp = mybir.ActivationFunctionType.Exp
    KPAD = KB * P

    nc = bacc.Bacc("TRN2", target_bir_lowering=False, debug=False)

    xT = nc.dram_tensor("xT", [H, S], bf16, kind="ExternalInput").ap()
    xkT = nc.dram_tensor("xkT", [H, KPAD], bf16, kind="ExternalInput").ap()
    # weights pre-packed host-side into the SBUF [p, t, f] layout so the
    # DMAs read contiguous multi-KB runs per partition (the (t p) f
    # rearrange reads 256-512B strided runs otherwise)
    wq0P = nc.dram_tensor("wq0P", [P, NHT * 2 * P], bf16, kind="ExternalInput").ap()
    wq1P = nc.dram_tensor("wq1P", [P, NHT * 2 * P], bf16, kind="ExternalInput").ap()
    wkP = nc.dram_tensor("wkP", [P, NHT * KF], bf16, kind="ExternalInput").ap()
    wvP = nc.dram_tensor("wvP", [P, NHT * KF], bf16, kind="ExternalInput").ap()
    woP = nc.dram_tensor("woP", [P, NMT * H], bf16, kind="ExternalInput").ap()
    mb = nc.dram_tensor("mb", [P, KB], f32, kind="ExternalInput").ap()
    outT = nc.dram_tensor("outT", [H, S], bf16, kind="ExternalOutput").ap()

    xT_r = xT.rearrange("(t p) s -> p t s", p=P)
    xkT_r = xkT.rearrange("(t p) s -> p t s", p=P)
    wq0P_r = wq0P.rearrange("p (t f) -> p t f", t=NHT)
    wq1P_r = wq1P.rearrange("p (t f) -> p t f", t=NHT)
    wkP_r = wkP.rearrange("p (t f) -> p t f", t=NHT)
    wvP_r = wvP.rearrange("p (t f) -> p t f", t=NHT)
    woP_r = woP.rearrange("p (t f) -> p t f", t=NMT)
    outT_r = outT.rearrange("(t p) s -> p t s", p=P)

    with tile.TileContext(nc) as tc, ExitStack() as es:
        ec = es.enter_context
        ec(nc.allow_low_precision(reason="bf16 matmuls, fp32 PSUM accum"))
        const_pool = ec(tc.tile_pool(name="const", bufs=1))
        wq_pool = ec(tc.tile_pool(name="wq", bufs=1))
        wo_pool = ec(tc.tile_pool(name="wo", bufs=1))
        wkv_pool = ec(tc.tile_pool(name="wkv", bufs=1))
        xt_pool = ec(tc.tile_pool(name="xt", bufs=2))
        xk_pool = ec(tc.tile_pool(name="xk", bufs=3))
        qt_pool = ec(tc.tile_pool(name="qt", bufs=1))
        kt_pool = ec(tc.tile_pool(name="kt", bufs=1))
        v_pool = ec(tc.tile_pool(name="vs", bufs=1))
        at_pool = ec(tc.tile_pool(name="at", bufs=2))
        e_pool = ec(tc.tile_pool(name="e", bufs=4))
        rc_pool = ec(tc.tile_pool(name="rc", bufs=2))
        rb_pool = ec(tc.tile_pool(name="rb", bufs=2))
        out_pool = ec(tc.tile_pool(name="outs", bufs=2))
        psa_pool = ec(tc.tile_pool(name="psa", bufs=2, space="PSUM"))
        po_pool = ec(tc.tile_pool(name="po", bufs=2, space="PSUM"))
        pp_pool = ec(tc.tile_pool(name="pp", bufs=2, space="PSUM"))

        # ---- constants ----
        mb_sb = const_pool.tile([P, KB], f32, tag="mb")
        # zero tiles for keep-warm dummy matmuls: the PE's HAM clock
        # gate re-throttles to 1.2GHz after >3.4us idle, so DMA-wait
        # gaps in the startup phase are bridged with throwaway matmuls
        dum_sb = const_pool.tile([P, P], bf16, tag="dum")
        dum2_sb = const_pool.tile([P, SQB], bf16, tag="dum2")
        nc.vector.memset(dum_sb, 0.0)
        nc.vector.memset(dum2_sb, 0.0)

        def warm(n):
            pd = po_pool.tile([P, SQB], f32, tag="po", name="warm")
            for i in range(n):
                nc.tensor.matmul(
                    pd, lhsT=dum_sb, rhs=dum2_sb,
                    start=(i == 0), stop=(i == n - 1),
                )

        # ---- k/v weights + gathered-key x, split fine-grained across
        # issuing engines so the first k-proj matmul starts early. Each
        # engine's DIRECT2D descriptor stream blocks when its ring fills,
        # so keep every engine's pre-attention queue shallow and ordered
        # critical-first (bulk weights ride behind on other queues).
        wk_sb = wkv_pool.tile([P, NHT, KF], bf16, tag="wk")
        wv_sb = wkv_pool.tile([P, NHT, KF], bf16, tag="wv")
        wq_sb = wq_pool.tile([P, NHT, QF], bf16, tag="wq")
        nkc = (KPAD + SQA - 1) // SQA
        xk_tiles = []
        # first xk chunk as TWO tiles (Tile's dependency granularity is
        # per-tile: one tile would make the first k-proj matmul wait for
        # ALL of it), each split across engines; xk0a-p0 leads the
        # scalar queue (the first k-proj matmul needs it + the small wk
        # piece on sync)
        kw0 = min(SQA, KPAD)
        xk0a = xk_pool.tile([P, 8, SQA], bf16, tag="xk0a", name="xk0a", bufs=1)
        xk0b = xk_pool.tile([P, 8, SQA], bf16, tag="xk0b", name="xk0b", bufs=1)
        xk_tiles.append((xk0a, xk0b))
        nc.sync.dma_start(out=wk_sb[:, 0:2, :], in_=wkP_r[:, 0:2, :])
        nc.scalar.dma_start(out=xk0a[:, 0:3, 0:kw0], in_=xkT_r[:, 0:3, 0:kw0])
        nc.sync.dma_start(out=xk0a[:, 3:6, 0:kw0], in_=xkT_r[:, 3:6, 0:kw0])
        nc.gpsimd.dma_start(out=xk0a[:, 6:8, 0:kw0], in_=xkT_r[:, 6:8, 0:kw0])
        nc.scalar.dma_start(out=wk_sb[:, 2:9, :], in_=wkP_r[:, 2:9, :])
        nc.scalar.dma_start(out=xk0b[:, 0:3, 0:kw0], in_=xkT_r[:, 8:11, 0:kw0])
        nc.sync.dma_start(out=xk0b[:, 3:6, 0:kw0], in_=xkT_r[:, 11:14, 0:kw0])
        nc.gpsimd.dma_start(out=xk0b[:, 6:8, 0:kw0], in_=xkT_r[:, 14:16, 0:kw0])
        nc.sync.dma_start(out=wk_sb[:, 9:16, :], in_=wkP_r[:, 9:16, :])
        nc.gpsimd.dma_start(out=wv_sb, in_=wvP_r)
        for c in range(1, nkc):
            k0 = c * SQA
            kw = min(SQA, KPAD - k0)
            xk = xk_pool.tile(
                [P, NHT, SQA], bf16, tag="xk", name=f"xk{c}", bufs=2
            )
            nc.sync.dma_start(out=xk[:, :, 0:kw], in_=xkT_r[:, :, k0:k0 + kw])
            xk_tiles.append(xk)

        def xk_src(c, ht, cols):
            if c == 0:
                a, b = xk_tiles[0]
                return a[:, ht, cols] if ht < 8 else b[:, ht - 8, cols]
            return xk_tiles[c][:, ht, cols]
        # mb (first needed by attention's exp) is tiny, rides early.
        # wq/wo/xt are BULK and not needed until ~25-90us: issuing them
        # now would steal DMA bandwidth from the critical kv-proj inputs
        # (the 16 DMA engines saturate at ~358GB/s). They are issued from
        # the (otherwise idle) gpsimd queue, gated behind kv-proj
        # progress via tiny copies that read kv-proj outputs.
        nc.gpsimd.dma_start(out=mb_sb, in_=mb)

        # kT stored twice: kta = [g0; g1] on partitions [0:64; 64:128],
        # ktb = [g1; g0] — so any (group, parity) pair can be read at
        # either partition base (matmul requires lhsT base == rhs base,
        # and the two parities row-tile concurrently).
        kta_sb = kt_pool.tile([P, KPAD], bf16, tag="kta")
        ktb_sb = kt_pool.tile([P, KPAD], bf16, tag="ktb")
        # per-group v tiles: [v (64) | ones (64)] per key block; the ones
        # BLOCK makes PV rows 64:128 the softmax denominator replicated
        # across 64 partitions — the PE does the broadcast for free, so
        # no gpsimd partition_broadcast is needed in the normalize chain.
        v_g = [
            v_pool.tile([P, KB, P], bf16, tag=f"v{g}", name=f"v{g}")
            for g in range(2)
        ]
        nc.vector.memset(v_g[0][:, :, 64:128], 1.0)
        nc.vector.memset(v_g[1][:, :, 64:128], 1.0)

        # ---- phase A1: k/v projections over kept keys ----
        warm(8)
        for c in range(nkc):
            if c > 0:
                warm(4)
            k0 = c * SQA
            kw = min(SQA, KPAD - k0)
            ps = pp_pool.tile([P, SQA], f32, tag="pp", name="ps_k")
            for ht in range(NHT):
                nc.tensor.matmul(
                    ps[:, 0:kw],
                    lhsT=wk_sb[:, ht, :],
                    rhs=xk_src(c, ht, slice(0, kw)),
                    start=(ht == 0),
                    stop=(ht == NHT - 1),
                )
            nc.scalar.copy(kta_sb[:, k0:k0 + kw], ps[:, 0:kw])
            nc.vector.tensor_copy(ktb_sb[0:64, k0:k0 + kw], ps[64:128, 0:kw])
            nc.vector.tensor_copy(ktb_sb[64:128, k0:k0 + kw], ps[0:64, 0:kw])
            # v (key-major): out[k, vf]
            for st in range(kw // P):
                kb = (k0 + st * P) // P
                psv = pp_pool.tile([P, KF], f32, tag="pp", name="ps_v")
                for ht in range(NHT):
                    nc.tensor.matmul(
                        psv,
                        lhsT=xk_src(c, ht, slice(st * P, (st + 1) * P)),
                        rhs=wv_sb[:, ht, :],
                        start=(ht == 0),
                        stop=(ht == NHT - 1),
                    )
                nc.scalar.copy(v_g[0][:, kb, 0:64], psv[:, 0:64])
                nc.scalar.copy(v_g[1][:, kb, 0:64], psv[:, 64:128])

        # wo streams in after xt0 (overlaps q-proj + attention start)
        wo_sb = wo_pool.tile([P, NMT, H], bf16, tag="wo")

        # ---- q projection, one q-tile at a time (xt DMA + wo after
        # tile 0's, interleaved into attention of the previous tile) ----
        # qT stored with col = (qtile, mt, q) so a head-column's q tile
        # is one contiguous SQB range per parity-partition half.
        qt_sb = qt_pool.tile([P, S * NMT], bf16, tag="qt")
        xt_tiles = {}

        def xt_dma(qt, eng=None):
            s0 = qt * SQB
            xt = xt_pool.tile([P, NHT, SQB], bf16, tag="xt", name=f"xt{qt}")
            (eng or nc.sync).dma_start(out=xt, in_=xT_r[:, :, s0:s0 + SQB])
            xt_tiles[qt] = xt

        def gate(dst, src):
            # tiny gpsimd copy writing INTO the destination tile of a
            # later bulk DMA: the write-after-write dependency delays
            # that DMA until `src` is ready (Tile's scheduler reorders
            # per-engine streams by dependency, so program order alone
            # cannot hold a dma_start back).
            nc.gpsimd.tensor_copy(dst, src)

        def qproj_mt(qt, mt):
            xt = xt_tiles[qt]
            ps = pp_pool.tile([P, SQB], f32, tag="pp", name="ps_q")
            for ht in range(NHT):
                nc.tensor.matmul(
                    ps,
                    lhsT=wq_sb[:, ht, mt * P:(mt + 1) * P],
                    rhs=xt[:, ht, :],
                    start=(ht == 0),
                    stop=(ht == NHT - 1),
                )
            c0 = (qt * NMT + mt) * SQB
            nc.vector.tensor_copy(qt_sb[:, c0:c0 + SQB], ps)

        def qproj_half(qt, half):
            qproj_mt(qt, 2 * half)
            qproj_mt(qt, 2 * half + 1)

        # bulk DMAs, gated: wq0+xt0 wait for k-proj chunk 0's kta copy
        # (~xk0 landed), wq1+xt1 for chunk 1's, wo for q-proj half 0
        # (needed only by out-proj ~100us in). Ungated they saturate the
        # DMA engines and starve the critical kv-proj inputs.
        gate(wq_sb[0:1, 0, 0:1], kta_sb[0:1, 0:1])
        nc.gpsimd.dma_start(out=wq_sb[:, :, 0:2 * P], in_=wq0P_r)
        xt0 = xt_pool.tile([P, NHT, SQB], bf16, tag="xt", name="xt0")
        xt_tiles[0] = xt0
        gate(xt0[0:1, 0, 0:1], kta_sb[0:1, 0:1])
        nc.gpsimd.dma_start(out=xt0, in_=xT_r[:, :, 0:SQB])
        gate(wq_sb[0:1, 0, 2 * P:2 * P + 1],
             kta_sb[0:1, min(1, nkc - 1) * SQA:min(1, nkc - 1) * SQA + 1])
        nc.gpsimd.dma_start(out=wq_sb[:, :, 2 * P:4 * P], in_=wq1P_r)
        xt1 = xt_pool.tile([P, NHT, SQB], bf16, tag="xt", name="xt1")
        xt_tiles[1] = xt1
        gate(xt1[0:1, 0, 0:1],
             kta_sb[0:1, min(1, nkc - 1) * SQA:min(1, nkc - 1) * SQA + 1])
        nc.gpsimd.dma_start(out=xt1, in_=xT_r[:, :, SQB:2 * SQB])
        warm(4)
        qproj_mt(0, 0)
        gate(wo_sb[0:1, 0, 0:1], qt_sb[0:1, 0:1])
        nc.gpsimd.dma_start(out=wo_sb, in_=woP_r)

        # ---- phase B/C: attention per q tile; q-proj of tile qt+1 and
        # out-projection of tile qt-1 fill the round boundaries ----
        # Round rnd = (g, j) handles head-column mt0+j of group g; the
        # two q-parities run as concurrent 64x128 row tiles.
        at_tiles = {}

        def outproj_8(qt, part, last=False):
            # output tiles batched 4-per-DMA: the SP engine needs ~0.9us
            # per dma_start issue, so 64 single-tile output DMAs cost more
            # SP serial time than the transfers themselves. In the final
            # call the PSUM->SBUF casts alternate vector/scalar (scalar's
            # exps are done) so the last output DMA isn't cast-serialized.
            q0 = qt * SQB
            for half in range(2):
                mt0 = part * 8 + 4 * half
                ob = out_pool.tile([P, 4, SQB], bf16, tag="ot", name="ob")
                for i in range(4):
                    mt = mt0 + i
                    ps = pp_pool.tile([P, SQB], f32, tag="pp", name="ps_o")
                    for kb4 in range(NMT):
                        nc.tensor.matmul(
                            ps,
                            lhsT=wo_sb[:, kb4, mt * P:(mt + 1) * P],
                            rhs=at_tiles[(qt, kb4)][:, :],
                            start=(kb4 == 0),
                            stop=(kb4 == NMT - 1),
                        )
                    eng = nc.scalar if (last and i % 2 == 0) else nc.vector
                    if eng is nc.scalar:
                        eng.copy(ob[:, i, :], ps)
                    else:
                        eng.tensor_copy(ob[:, i, :], ps)
                nc.sync.dma_start(
                    out=outT_r[:, mt0:mt0 + 4, q0:q0 + SQB], in_=ob
                )

        def outproj_units(qt):
            # out-projection of tile qt as 16 small units (one output
            # column each: 4 accumulating MMs + a PSUM->SBUF cast; the
            # 4th unit of a group issues the batched output DMA) so they
            # drain INSIDE the next tile's kb loop, overlapping the
            # exp-paced stretches instead of serializing after them.
            q0 = qt * SQB
            obs = {}
            units = []
            for grp in range(4):
                for i in range(4):
                    def col(grp=grp, i=i):
                        if i == 0:
                            obs[grp] = out_pool.tile(
                                [P, 4, SQB], bf16, tag="ot", name="ob"
                            )
                        mt = grp * 4 + i
                        ps = pp_pool.tile([P, SQB], f32, tag="pp", name="ps_o")
                        for kb4 in range(NMT):
                            nc.tensor.matmul(
                                ps,
                                lhsT=wo_sb[:, kb4, mt * P:(mt + 1) * P],
                                rhs=at_tiles[(qt, kb4)][:, :],
                                start=(kb4 == 0),
                                stop=(kb4 == NMT - 1),
                            )
                        nc.vector.tensor_copy(obs[grp][:, i, :], ps)
                        if i == 3:
                            m0 = grp * 4
                            nc.sync.dma_start(
                                out=outT_r[:, m0:m0 + 4, q0:q0 + SQB],
                                in_=obs[grp],
                            )
                    units.append(col)
            return units

        def qproj_units(qt, mt):
            # one q-proj head-column as two 8-deep half units. The two
            # halves share an open PSUM accumulation, so they must stay
            # adjacent in the drain order with no other pp-pool user
            # between them (only used for qt0, where no outproj runs).
            state = {}

            def half(h):
                if h == 0:
                    state["ps"] = pp_pool.tile(
                        [P, SQB], f32, tag="pp", name="ps_q"
                    )
                ps = state["ps"]
                xt = xt_tiles[qt]
                for ht in range(8 * h, 8 * h + 8):
                    nc.tensor.matmul(
                        ps,
                        lhsT=wq_sb[:, ht, mt * P:(mt + 1) * P],
                        rhs=xt[:, ht, :],
                        start=(ht == 0),
                        stop=(ht == NHT - 1),
                    )
                if h == 1:
                    c0 = (qt * NMT + mt) * SQB
                    nc.vector.tensor_copy(qt_sb[:, c0:c0 + SQB], ps)

            return [lambda: half(0), lambda: half(1)]

        for qt in range(NQT):
            for mt in range(NMT):
                at_tiles[(qt, mt)] = at_pool.tile(
                    [P, SQB], bf16, tag=f"at{mt}", name=f"at_{qt}_{mt}"
                )
            if qt == 0:
                smalls = []
                for (q, m) in [(0, 1), (0, 2), (0, 3),
                               (1, 0), (1, 1), (1, 2), (1, 3)]:
                    smalls += qproj_units(q, m)
                smalls.append(lambda: xt_dma(2))
            else:
                smalls = outproj_units(qt - 1)
            si = 0
            for rnd in range(4):
                g = rnd // 2
                j = rnd % 2
                mt0 = 2 * g
                # group g's keys at partition base 0 (ktA) and 64 (ktB)
                ktA = kta_sb if g == 0 else ktb_sb
                ktB = ktb_sb if g == 0 else kta_sb
                col0 = (qt * NMT + mt0 + j) * SQB
                po = [
                    po_pool.tile([P, SQB], f32, tag="po", name=f"po{p}")
                    for p in range(2)
                ]
                for kk in range(0, KB, 2):
                    kbs = [kb for kb in (kk, kk + 1) if kb < KB]
                    # scores burst (64-row mode): both parities of up to
                    # two key blocks; parity pairs run concurrently on
                    # PE row-tiles (0,0)/(64,0), separate PSUM banks.
                    pss = []
                    for kb in kbs:
                        ps = psa_pool.tile([P, 2 * SQB], f32, tag="ps")
                        nc.tensor.matmul(
                            ps[:, 0:SQB],
                            lhsT=ktA[0:D, kb * P:(kb + 1) * P],
                            rhs=qt_sb[0:D, col0:col0 + SQB],
                            start=True,
                            stop=True,
                        )
                        nc.tensor.matmul(
                            ps[:, SQB:2 * SQB],
                            lhsT=ktB[D:P, kb * P:(kb + 1) * P],
                            rhs=qt_sb[D:P, col0:col0 + SQB],
                            start=True,
                            stop=True,
                        )
                        pss.append(ps)
                    ees = []
                    for ps, kb in zip(pss, kbs):
                        e = e_pool.tile([P, 2 * SQB], bf16, tag="e")
                        nc.scalar.activation(
                            e, ps, Exp,
                            bias=mb_sb[:, kb:kb + 1], scale=float(SCALE),
                        )
                        ees.append(e)
                    # PV burst (128-row mode); both parities share the
                    # v stationary per key block.
                    for e, kb in zip(ees, kbs):
                        for p in range(2):
                            nc.tensor.matmul(
                                po[p],
                                lhsT=v_g[g][:, kb, :],
                                rhs=e[:, p * SQB:(p + 1) * SQB],
                                start=(kb == 0),
                                stop=(kb == KB - 1),
                            )
                    # drain one fill unit (same 128-row mode as PV)
                    if si < len(smalls):
                        smalls[si]()
                        si += 1
                # normalize: rows 0..63 are numerator^T, rows 64..127 the
                # denominator replicated; copy to base-0 partitions (the
                # custom-DVE recip mis-reads nonzero partition bases),
                # reciprocal, then per-parity multiply. All on DVE.
                dn = rc_pool.tile([D, 2 * SQB], f32, tag="dn")
                for p in range(2):
                    nc.vector.tensor_copy(
                        dn[:, p * SQB:(p + 1) * SQB], po[p][64:128, :]
                    )
                rc = rb_pool.tile([D, 2 * SQB], f32, tag="rc")
                nc.vector.reciprocal_approx_fast(rc, dn)
                for p in range(2):
                    nc.vector.tensor_mul(
                        at_tiles[(qt, mt0 + j)][p * D:(p + 1) * D, :],
                        po[p][0:64, :],
                        rc[:, p * SQB:(p + 1) * SQB],
                    )
                # round boundary: next tile's q-proj head-column (middle
                # tiles; qt0's q-proj and all out-proj drain in-loop)
                if 0 < qt < NQT - 1:
                    if rnd == 2 and qt + 2 < NQT:
                        xt_dma(qt + 2)
                    qproj_mt(qt + 1, rnd)
                if rnd == 3:
                    while si < len(smalls):
                        smalls[si]()
                        si += 1
        for part in range(2):
            outproj_8(NQT - 1, part, last=True)
    nc.compile()
    return nc


_NC_CACHE = {}


def _get_nc(KB):
    if KB not in _NC_CACHE:
        _NC_CACHE[KB] = _build_bass(KB)
    return _NC_CACHE[KB]


def _make_in_maps(inputs):
    import ml_dtypes

    bf = ml_dtypes.bfloat16
    x = np.asarray(inputs["x"], dtype=np.float32)
    mask = np.asarray(inputs["mask"])
    Wq = np.asarray(inputs["Wq"], dtype=np.float32)
    Wk = np.asarray(inputs["Wk"], dtype=np.float32)
    Wv = np.asarray(inputs["Wv"], dtype=np.float32)
    Wo = np.asarray(inputs["Wo"], dtype=np.float32)

    # gather kept (unmasked) key positions per batch; pad to common KPAD
    idxs = [np.nonzero(mask[b, 0, 0, 0, :] != 0)[0] for b in range(B)]
    kept_max = max(1, max(len(i) for i in idxs))
    KB = (kept_max + P - 1) // P
    KPAD = KB * P

    xTs, xkTs, mbs = [], [], []
    for b in range(B):
        xb = x[b].astype(bf)
        xTs.append(np.ascontiguousarray(xb.T))
        xk = np.zeros((KPAD, H), dtype=bf)
        xk[: len(idxs[b])] = xb[idxs[b]]
        xkTs.append(np.ascontiguousarray(xk.T))
        bias = np.full(KPAD, np.float32(MASK_NEG), dtype=np.float32)
        bias[: len(idxs[b])] = 0.0
        mbs.append(np.ascontiguousarray(bias.reshape(KB, P).T))

    in_maps = []
    for c in range(NCORES):
        b, gp = divmod(c, 4)
        qs = slice(gp * QF, (gp + 1) * QF)
        ks = slice(gp * KF, (gp + 1) * KF)
        def pack(wT, F):
            # (H-or-QF, F) -> [P, T*F] in the SBUF [p, t, f] layout
            T = wT.shape[0] // P
            return np.ascontiguousarray(
                wT.reshape(T, P, F).transpose(1, 0, 2).reshape(P, T * F)
            )

        wqT = Wq[qs, :].T.astype(bf)
        in_maps.append({
            "xT": xTs[b],
            "xkT": xkTs[b],
            "wq0P": pack(wqT[:, 0:2 * P], 2 * P),
            "wq1P": pack(wqT[:, 2 * P:4 * P], 2 * P),
            "wkP": pack(Wk[ks, :].T.astype(bf), KF),
            "wvP": pack(Wv[ks, :].T.astype(bf), KF),
            "woP": pack(Wo[:, qs].T.astype(bf), H),
            "mb": mbs[b],
        })
    return in_maps, KB


def kernel(**inputs):
    from concourse.bass_utils import run_bass_kernel_spmd

    in_maps, KB = _make_in_maps(inputs)
    nc = _get_nc(KB)
    res = run_bass_kernel_spmd(nc, in_maps, core_ids=list(range(NCORES)))
    outs = [np.asarray(r["outT"], dtype=np.float32) for r in res.results]
    out = np.empty((B, S, H), dtype=np.float32)
    for b in range(B):
        acc = outs[4 * b] + outs[4 * b + 1] + outs[4 * b + 2] + outs[4 * b + 3]
        out[b] = acc.T
    return out


# revision 37
# speedup vs baseline: 1.0244x; 1.0244x over previous
"""GQA kernel for 8 Trainium2 NeuronCores.

Problem: nn_GroupQueryAttention — B=2, S=2048, HIDDEN=2048, 32 heads,
8 kv-groups, head_dim 64.

Sharding: data parallel on batch (2) x tensor parallel on kv-groups (4
group-pairs). Core c owns batch c//4 and kv-groups {2*(c%4), 2*(c%4)+1}
(512 q-features, 128 kv-features). Each core computes a partial
out-projection (Wo columns of its features); host sums 4 partials per
batch.

Key optimizations over the fp32r baseline:
  - all matmul operands in bf16 (fp32r ran under a HW power throttle at
    ~1.2GHz; bf16 streams 1 col/cycle at full clock). PSUM accumulation
    stays fp32.
  - key compaction: the mask is per-key (broadcast over queries+heads),
    so masked keys contribute nothing anywhere. Host gathers the
    unmasked key positions, pads to a multiple of 128, and the kernel
    only projects/attends over the kept keys (~half of 2048 for the
    random mask). Padded key rows get an exp bias of -30000 so E=0.
  - scores matmuls are 64-deep (head_dim): the two q-parities of a head
    column live on disjoint SBUF partition halves (kta=[g0;g1],
    ktb=[g1;g0]), so their matmuls run CONCURRENTLY as 64x128 row tiles
    (tile_position (0,0) / (64,0)) — 2x scores throughput. Scores for
    two key blocks are batched per row-mode burst to halve the 64<->128
    row-mode switch drains.
  - one exp + one normalize chain per (head-column, key-block):
    [128, 1024] activation covering both parities (same key-block bias).
  - k/v projections run first behind the small wk/wv DMAs (wk + first
    xk chunk split fine and issued from different engines so the PE
    starts early); wq/wo stream during kv-proj.
  - q-proj of tile qt+1 and out-projection of tile qt-1 are interleaved
    into tile qt's attention round boundaries to fill normalize bubbles.
    qproj PSUM->SBUF copies go on the vector engine (scalar is
    exp-saturated during attention).
  - softmax division: PV's stationary columns 64:128 are all ones, so
    the PE replicates the denominator across 64 PSUM partitions for
    free; the normalize chain is copy -> approx-reciprocal -> multiply,
    all on DVE (no gpsimd partition_broadcast).
  - bf16 output partials summed on host in fp32.
"""

import numpy as np

B = 2
S = 2048
H = 2048
G = 8            # kv groups total
HPG = 4          # heads per group
D = 64           # head dim
NCORES = 8
QF = 512         # q features per core (2 groups * 4 heads * 64)
KF = 128         # kv features per core (2 groups * 64)
SCALE = 1.0 / np.sqrt(np.float32(D))
P = 128
SQA = 512        # seq chunk for projection phase (moving dim)
SQB = 512        # q tile for attention / out-proj phase
NHT = H // P     # 16 hidden partition tiles
NMT = QF // P    # 4 q-feature partition tiles
NQT = S // SQB   # 4 q tiles
MASK_NEG = -30000.0


def _build_bass(KB):
    """Build the per-core program for KPAD = KB*128 kept+padded keys."""
    from contextlib import ExitStack

    import concourse.tile as tile
    from concourse import bacc, mybir

    f32 = mybir.dt.float32
    bf16 = mybir.dt.bfloat16
    Exp = mybir.ActivationFunctionType.Exp
    KPAD = KB * P

    nc = bacc.Bacc("TRN2", target_bir_lowering=False, debug=False)

    xT = nc.dram_tensor("xT", [H, S], bf16, kind="ExternalInput").ap()
    xkT = nc.dram_tensor("xkT", [H, KPAD], bf16, kind="ExternalInput").ap()
    # weights pre-packed host-side into the SBUF [p, t, f] layout so the
    # DMAs read contiguous multi-KB runs per partition (the (t p) f
    # rearrange reads 256-512B strided runs otherwise)
    wq0P = nc.dram_tensor("wq0P", [P, NHT * 2 * P], bf16, kind="ExternalInput").ap()
    wq1P = nc.dram_tensor("wq1P", [P, NHT * 2 * P], bf16, kind="ExternalInput").ap()
    wkP = nc.dram_tensor("wkP", [P, NHT * KF], bf16, kind="ExternalInput").ap()
    wvP = nc.dram_tensor("wvP", [P, NHT * KF], bf16, kind="ExternalInput").ap()
    woP = nc.dram_tensor("woP", [P, NMT * H], bf16, kind="ExternalInput").ap()
    mb = nc.dram_tensor("mb", [P, KB], f32, kind="ExternalInput").ap()
    outT = nc.dram_tensor("outT", [H, S], bf16, kind="ExternalOutput").ap()

    xT_r = xT.rearrange("(t p) s -> p t s", p=P)
    xkT_r = xkT.rearrange("(t p) s -> p t s", p=P)
    wq0P_r = wq0P.rearrange("p (t f) -> p t f", t=NHT)
    wq1P_r = wq1P.rearrange("p (t f) -> p t f", t=NHT)
    wkP_r = wkP.rearrange("p (t f) -> p t f", t=NHT)
    wvP_r = wvP.rearrange("p (t f) -> p t f", t=NHT)
    woP_r = woP.rearrange("p (t f) -> p t f", t=NMT)
    outT_r = outT.rearrange("(t p) s -> p t s", p=P)

    with tile.TileContext(nc) as tc, ExitStack() as es:
        ec = es.enter_context
        ec(nc.allow_low_precision(reason="bf16 matmuls, fp32 PSUM accum"))
        const_pool = ec(tc.tile_pool(name="const", bufs=1))
        wq_pool = ec(tc.tile_pool(name="wq", bufs=1))
        wo_pool = ec(tc.tile_pool(name="wo", bufs=1))
        wkv_pool = ec(tc.tile_pool(name="wkv", bufs=1))
        xt_pool = ec(tc.tile_pool(name="xt", bufs=2))
        xk_pool = ec(tc.tile_pool(name="xk", bufs=3))
        qt_pool = ec(tc.tile_pool(name="qt", bufs=1))
        kt_pool = ec(tc.tile_pool(name="kt", bufs=1))
        v_pool = ec(tc.tile_pool(name="vs", bufs=1))
        at_pool = ec(tc.tile_pool(name="at", bufs=2))
        e_pool = ec(tc.tile_pool(name="e", bufs=4))
        rc_pool = ec(tc.tile_pool(name="rc", bufs=2))
        rb_pool = ec(tc.tile_pool(name="rb", bufs=2))
        out_pool = ec(tc.tile_pool(name="outs", bufs=2))
        psa_pool = ec(tc.tile_pool(name="psa", bufs=2, space="PSUM"))
        po_pool = ec(tc.tile_pool(name="po", bufs=2, space="PSUM"))
        pp_pool = ec(tc.tile_pool(name="pp", bufs=2, space="PSUM"))

        # ---- constants ----
        mb_sb = const_pool.tile([P, KB], f32, tag="mb")
        # zero tiles for keep-warm dummy matmuls: the PE's HAM clock
        # gate re-throttles to 1.2GHz after >3.4us idle, so DMA-wait
        # gaps in the startup phase are bridged with throwaway matmuls
        dum_sb = const_pool.tile([P, P], bf16, tag="dum")
        dum2_sb = const_pool.tile([P, SQB], bf16, tag="dum2")
        nc.vector.memset(dum_sb, 0.0)
        nc.vector.memset(dum2_sb, 0.0)

        def warm(n):
            pd = po_pool.tile([P, SQB], f32, tag="po", name="warm")
            for i in range(n):
                nc.tensor.matmul(
                    pd, lhsT=dum_sb, rhs=dum2_sb,
                    start=(i == 0), stop=(i == n - 1),
                )

        # ---- k/v weights + gathered-key x, split fine-grained across
        # issuing engines so the first k-proj matmul starts early. Each
        # engine's DIRECT2D descriptor stream blocks when its ring fills,
        # so keep every engine's pre-attention queue shallow and ordered
        # critical-first (bulk weights ride behind on other queues).
        wk_sb = wkv_pool.tile([P, NHT, KF], bf16, tag="wk")
        wv_sb = wkv_pool.tile([P, NHT, KF], bf16, tag="wv")
        wq_sb = wq_pool.tile([P, NHT, QF], bf16, tag="wq")
        nkc = (KPAD + SQA - 1) // SQA
        xk_tiles = []
        # first xk chunk split 4-way across engines so the PE starts
        # after a quarter of it; xk0-q0 leads the scalar queue (the
        # first k-proj matmul needs it + the small wk piece on sync)
        xk0 = xk_pool.tile([P, NHT, SQA], bf16, tag="xk", name="xk0")
        xk_tiles.append(xk0)
        kw0 = min(SQA, KPAD)
        nc.sync.dma_start(out=wk_sb[:, 0:2, :], in_=wkP_r[:, 0:2, :])
        for hq in range(8):
            eng = [nc.scalar, nc.sync, nc.gpsimd,
                   nc.sync, nc.scalar, nc.sync, nc.gpsimd, nc.sync][hq]
            eng.dma_start(
                out=xk0[:, 2 * hq:2 * hq + 2, 0:kw0],
                in_=xkT_r[:, 2 * hq:2 * hq + 2, 0:kw0],
            )
        nc.scalar.dma_start(out=wk_sb[:, 2:9, :], in_=wkP_r[:, 2:9, :])
        nc.sync.dma_start(out=wk_sb[:, 9:16, :], in_=wkP_r[:, 9:16, :])
        nc.gpsimd.dma_start(out=wv_sb, in_=wvP_r)
        for c in range(1, nkc):
            k0 = c * SQA
            kw = min(SQA, KPAD - k0)
            xk = xk_pool.tile([P, NHT, SQA], bf16, tag="xk", name=f"xk{c}")
            nc.sync.dma_start(out=xk[:, :, 0:kw], in_=xkT_r[:, :, k0:k0 + kw])
            xk_tiles.append(xk)
        # mb (first needed by attention's exp) is tiny, rides early.
        # wq/wo/xt are BULK and not needed until ~25-90us: issuing them
        # now would steal DMA bandwidth from the critical kv-proj inputs
        # (the 16 DMA engines saturate at ~358GB/s). They are issued from
        # the (otherwise idle) gpsimd queue, gated behind kv-proj
        # progress via tiny copies that read kv-proj outputs.
        nc.gpsimd.dma_start(out=mb_sb, in_=mb)

        # kT stored twice: kta = [g0; g1] on partitions [0:64; 64:128],
        # ktb = [g1; g0] — so any (group, parity) pair can be read at
        # either partition base (matmul requires lhsT base == rhs base,
        # and the two parities row-tile concurrently).
        kta_sb = kt_pool.tile([P, KPAD], bf16, tag="kta")
        ktb_sb = kt_pool.tile([P, KPAD], bf16, tag="ktb")
        # per-group v tiles: [v (64) | ones (64)] per key block; the ones
        # BLOCK makes PV rows 64:128 the softmax denominator replicated
        # across 64 partitions — the PE does the broadcast for free, so
        # no gpsimd partition_broadcast is needed in the normalize chain.
        v_g = [
            v_pool.tile([P, KB, P], bf16, tag=f"v{g}", name=f"v{g}")
            for g in range(2)
        ]
        nc.vector.memset(v_g[0][:, :, 64:128], 1.0)
        nc.vector.memset(v_g[1][:, :, 64:128], 1.0)

        # ---- phase A1: k/v projections over kept keys ----
        warm(8)
        for c in range(nkc):
            if c > 0:
                warm(4)
            k0 = c * SQA
            kw = min(SQA, KPAD - k0)
            xk = xk_tiles[c]
            ps = pp_pool.tile([P, SQA], f32, tag="pp", name="ps_k")
            for ht in range(NHT):
                nc.tensor.matmul(
                    ps[:, 0:kw],
                    lhsT=wk_sb[:, ht, :],
                    rhs=xk[:, ht, 0:kw],
                    start=(ht == 0),
                    stop=(ht == NHT - 1),
                )
            nc.scalar.copy(kta_sb[:, k0:k0 + kw], ps[:, 0:kw])
            nc.vector.tensor_copy(ktb_sb[0:64, k0:k0 + kw], ps[64:128, 0:kw])
            nc.vector.tensor_copy(ktb_sb[64:128, k0:k0 + kw], ps[0:64, 0:kw])
            # v (key-major): out[k, vf]
            for st in range(kw // P):
                kb = (k0 + st * P) // P
                psv = pp_pool.tile([P, KF], f32, tag="pp", name="ps_v")
                for ht in range(NHT):
                    nc.tensor.matmul(
                        psv,
                        lhsT=xk[:, ht, st * P:(st + 1) * P],
                        rhs=wv_sb[:, ht, :],
                        start=(ht == 0),
                        stop=(ht == NHT - 1),
                    )
                nc.scalar.copy(v_g[0][:, kb, 0:64], psv[:, 0:64])
                nc.scalar.copy(v_g[1][:, kb, 0:64], psv[:, 64:128])

        # wo streams in after xt0 (overlaps q-proj + attention start)
        wo_sb = wo_pool.tile([P, NMT, H], bf16, tag="wo")

        # ---- q projection, one q-tile at a time (xt DMA + wo after
        # tile 0's, interleaved into attention of the previous tile) ----
        # qT stored with col = (qtile, mt, q) so a head-column's q tile
        # is one contiguous SQB range per parity-partition half.
        qt_sb = qt_pool.tile([P, S * NMT], bf16, tag="qt")
        xt_tiles = {}

        def xt_dma(qt, eng=None):
            s0 = qt * SQB
            xt = xt_pool.tile([P, NHT, SQB], bf16, tag="xt", name=f"xt{qt}")
            (eng or nc.sync).dma_start(out=xt, in_=xT_r[:, :, s0:s0 + SQB])
            xt_tiles[qt] = xt

        def gate(dst, src):
            # tiny gpsimd copy writing INTO the destination tile of a
            # later bulk DMA: the write-after-write dependency delays
            # that DMA until `src` is ready (Tile's scheduler reorders
            # per-engine streams by dependency, so program order alone
            # cannot hold a dma_start back).
            nc.gpsimd.tensor_copy(dst, src)

        def qproj_mt(qt, mt):
            xt = xt_tiles[qt]
            ps = pp_pool.tile([P, SQB], f32, tag="pp", name="ps_q")
            for ht in range(NHT):
                nc.tensor.matmul(
                    ps,
                    lhsT=wq_sb[:, ht, mt * P:(mt + 1) * P],
                    rhs=xt[:, ht, :],
                    start=(ht == 0),
                    stop=(ht == NHT - 1),
                )
            c0 = (qt * NMT + mt) * SQB
            nc.vector.tensor_copy(qt_sb[:, c0:c0 + SQB], ps)

        def qproj_half(qt, half):
            qproj_mt(qt, 2 * half)
            qproj_mt(qt, 2 * half + 1)

        # bulk DMAs, gated: wq0+xt0 wait for k-proj chunk 0's kta copy
        # (~xk0 landed), wq1+xt1 for chunk 1's, wo for q-proj half 0
        # (needed only by out-proj ~100us in). Ungated they saturate the
        # DMA engines and starve the critical kv-proj inputs.
        gate(wq_sb[0:1, 0, 0:1], kta_sb[0:1, 0:1])
        nc.gpsimd.dma_start(out=wq_sb[:, :, 0:2 * P], in_=wq0P_r)
        xt0 = xt_pool.tile([P, NHT, SQB], bf16, tag="xt", name="xt0")
        xt_tiles[0] = xt0
        gate(xt0[0:1, 0, 0:1], kta_sb[0:1, 0:1])
        nc.gpsimd.dma_start(out=xt0, in_=xT_r[:, :, 0:SQB])
        gate(wq_sb[0:1, 0, 2 * P:2 * P + 1],
             kta_sb[0:1, min(1, nkc - 1) * SQA:min(1, nkc - 1) * SQA + 1])
        nc.gpsimd.dma_start(out=wq_sb[:, :, 2 * P:4 * P], in_=wq1P_r)
        xt1 = xt_pool.tile([P, NHT, SQB], bf16, tag="xt", name="xt1")
        xt_tiles[1] = xt1
        gate(xt1[0:1, 0, 0:1],
             kta_sb[0:1, min(1, nkc - 1) * SQA:min(1, nkc - 1) * SQA + 1])
        nc.gpsimd.dma_start(out=xt1, in_=xT_r[:, :, SQB:2 * SQB])
        warm(4)
        qproj_mt(0, 0)
        gate(wo_sb[0:1, 0, 0:1], qt_sb[0:1, 0:1])
        nc.gpsimd.dma_start(out=wo_sb, in_=woP_r)

        # ---- phase B/C: attention per q tile; q-proj of tile qt+1 and
        # out-projection of tile qt-1 fill the round boundaries ----
        # Round rnd = (g, j) handles head-column mt0+j of group g; the
        # two q-parities run as concurrent 64x128 row tiles.
        at_tiles = {}

        def outproj_4(qt, grp, last=False):
            # one output group: 4 head-columns batched into one DMA (the
            # SP engine needs ~0.9us per dma_start issue, so 64
            # single-tile output DMAs would cost more SP serial time
            # than the transfers themselves). In the final calls the
            # PSUM->SBUF casts alternate vector/scalar (scalar's exps
            # are done) and the DMA goes via the idle gpsimd queue so
            # the tail isn't serialized behind the sync-engine barrier.
            q0 = qt * SQB
            mt0 = grp * 4
            ob = out_pool.tile([P, 4, SQB], bf16, tag="ot", name="ob")
            for i in range(4):
                mt = mt0 + i
                ps = pp_pool.tile([P, SQB], f32, tag="pp", name="ps_o")
                for kb4 in range(NMT):
                    nc.tensor.matmul(
                        ps,
                        lhsT=wo_sb[:, kb4, mt * P:(mt + 1) * P],
                        rhs=at_tiles[(qt, kb4)][:, :],
                        start=(kb4 == 0),
                        stop=(kb4 == NMT - 1),
                    )
                eng = nc.scalar if (last and i % 2 == 0) else nc.vector
                if eng is nc.scalar:
                    eng.copy(ob[:, i, :], ps)
                else:
                    eng.tensor_copy(ob[:, i, :], ps)
            (nc.gpsimd if last else nc.sync).dma_start(
                out=outT_r[:, mt0:mt0 + 4, q0:q0 + SQB], in_=ob
            )

        def outproj_8(qt, part, last=False):
            outproj_4(qt, 2 * part, last=last)
            outproj_4(qt, 2 * part + 1, last=last)

        for qt in range(NQT):
            for mt in range(NMT):
                at_tiles[(qt, mt)] = at_pool.tile(
                    [P, SQB], bf16, tag=f"at{mt}", name=f"at_{qt}_{mt}"
                )
            for rnd in range(4):
                g = rnd // 2
                j = rnd % 2
                mt0 = 2 * g
                # group g's keys at partition base 0 (ktA) and 64 (ktB)
                ktA = kta_sb if g == 0 else ktb_sb
                ktB = ktb_sb if g == 0 else kta_sb
                col0 = (qt * NMT + mt0 + j) * SQB
                po = [
                    po_pool.tile([P, SQB], f32, tag="po", name=f"po{p}")
                    for p in range(2)
                ]
                for kk in range(0, KB, 2):
                    kbs = [kb for kb in (kk, kk + 1) if kb < KB]
                    # scores burst (64-row mode): both parities of up to
                    # two key blocks; parity pairs run concurrently on
                    # PE row-tiles (0,0)/(64,0), separate PSUM banks.
                    pss = []
                    for kb in kbs:
                        ps = psa_pool.tile([P, 2 * SQB], f32, tag="ps")
                        nc.tensor.matmul(
                            ps[:, 0:SQB],
                            lhsT=ktA[0:D, kb * P:(kb + 1) * P],
                            rhs=qt_sb[0:D, col0:col0 + SQB],
                            start=True,
                            stop=True,
                        )
                        nc.tensor.matmul(
                            ps[:, SQB:2 * SQB],
                            lhsT=ktB[D:P, kb * P:(kb + 1) * P],
                            rhs=qt_sb[D:P, col0:col0 + SQB],
                            start=True,
                            stop=True,
                        )
                        pss.append(ps)
                    ees = []
                    for ps, kb in zip(pss, kbs):
                        e = e_pool.tile([P, 2 * SQB], bf16, tag="e")
                        nc.scalar.activation(
                            e, ps, Exp,
                            bias=mb_sb[:, kb:kb + 1], scale=float(SCALE),
                        )
                        ees.append(e)
                    # PV burst (128-row mode); both parities share the
                    # v stationary per key block.
                    for e, kb in zip(ees, kbs):
                        for p in range(2):
                            nc.tensor.matmul(
                                po[p],
                                lhsT=v_g[g][:, kb, :],
                                rhs=e[:, p * SQB:(p + 1) * SQB],
                                start=(kb == 0),
                                stop=(kb == KB - 1),
                            )
                # normalize: rows 0..63 are numerator^T, rows 64..127 the
                # denominator replicated; copy to base-0 partitions (the
                # custom-DVE recip mis-reads nonzero partition bases),
                # reciprocal, then per-parity multiply. All on DVE.
                dn = rc_pool.tile([D, 2 * SQB], f32, tag="dn")
                for p in range(2):
                    nc.vector.tensor_copy(
                        dn[:, p * SQB:(p + 1) * SQB], po[p][64:128, :]
                    )
                rc = rb_pool.tile([D, 2 * SQB], f32, tag="rc")
                nc.vector.reciprocal_approx_fast(rc, dn)
                for p in range(2):
                    nc.vector.tensor_mul(
                        at_tiles[(qt, mt0 + j)][p * D:(p + 1) * D, :],
                        po[p][0:64, :],
                        rc[:, p * SQB:(p + 1) * SQB],
                    )
                # fill the round boundary with independent PE work
                # (qt0 attention starts right after q-proj of mt0 alone;
                # each boundary then supplies the next rounds' q blocks)
                if qt == 0:
                    if rnd == 0:
                        qproj_mt(0, 1)
                        qproj_mt(0, 2)
                    elif rnd == 1:
                        qproj_mt(0, 3)
                        qproj_mt(1, 0)
                    elif rnd == 2:
                        qproj_mt(1, 1)
                        qproj_mt(1, 2)
                    else:
                        qproj_mt(1, 3)
                        xt_dma(2)
                elif qt < NQT - 1:
                    if rnd <= 1:
                        qproj_half(qt + 1, rnd)
                    else:
                        if rnd == 2 and qt + 2 < NQT:
                            xt_dma(qt + 2)
                        outproj_8(qt - 1, rnd - 2)
                else:
                    # last tile: spread the previous tile's out-proj one
                    # group per boundary — its rounds are exp-paced with
                    # ~3us of PE slack each, so 2-group boundaries would
                    # overpack rounds 2/3 while 0/1 idle
                    outproj_4(qt - 1, rnd)
        for part in range(2):
            outproj_8(NQT - 1, part, last=True)
    nc.compile()
    return nc


_NC_CACHE = {}


def _get_nc(KB):
    if KB not in _NC_CACHE:
        _NC_CACHE[KB] = _build_bass(KB)
    return _NC_CACHE[KB]


def _make_in_maps(inputs):
    import ml_dtypes

    bf = ml_dtypes.bfloat16
    x = np.asarray(inputs["x"], dtype=np.float32)
    mask = np.asarray(inputs["mask"])
    Wq = np.asarray(inputs["Wq"], dtype=np.float32)
    Wk = np.asarray(inputs["Wk"], dtype=np.float32)
    Wv = np.asarray(inputs["Wv"], dtype=np.float32)
    Wo = np.asarray(inputs["Wo"], dtype=np.float32)

    # gather kept (unmasked) key positions per batch; pad to common KPAD
    idxs = [np.nonzero(mask[b, 0, 0, 0, :] != 0)[0] for b in range(B)]
    kept_max = max(1, max(len(i) for i in idxs))
    KB = (kept_max + P - 1) // P
    KPAD = KB * P

    xTs, xkTs, mbs = [], [], []
    for b in range(B):
        xb = x[b].astype(bf)
        xTs.append(np.ascontiguousarray(xb.T))
        xk = np.zeros((KPAD, H), dtype=bf)
        xk[: len(idxs[b])] = xb[idxs[b]]
        xkTs.append(np.ascontiguousarray(xk.T))
        bias = np.full(KPAD, np.float32(MASK_NEG), dtype=np.float32)
        bias[: len(idxs[b])] = 0.0
        mbs.append(np.ascontiguousarray(bias.reshape(KB, P).T))

    in_maps = []
    for c in range(NCORES):
        b, gp = divmod(c, 4)
        qs = slice(gp * QF, (gp + 1) * QF)
        ks = slice(gp * KF, (gp + 1) * KF)
        def pack(wT, F):
            # (H-or-QF, F) -> [P, T*F] in the SBUF [p, t, f] layout
            T = wT.shape[0] // P
            return np.ascontiguousarray(
                wT.reshape(T, P, F).transpose(1, 0, 2).reshape(P, T * F)
            )

        wqT = Wq[qs, :].T.astype(bf)
        in_maps.append({
            "xT": xTs[b],
            "xkT": xkTs[b],
            "wq0P": pack(wqT[:, 0:2 * P], 2 * P),
            "wq1P": pack(wqT[:, 2 * P:4 * P], 2 * P),
            "wkP": pack(Wk[ks, :].T.astype(bf), KF),
            "wvP": pack(Wv[ks, :].T.astype(bf), KF),
            "woP": pack(Wo[:, qs].T.astype(bf), H),
            "mb": mbs[b],
        })
    return in_maps, KB


def kernel(**inputs):
    from concourse.bass_utils import run_bass_kernel_spmd

    in_maps, KB = _make_in_maps(inputs)
    nc = _get_nc(KB)
    res = run_bass_kernel_spmd(nc, in_maps, core_ids=list(range(NCORES)))
    outs = [np.asarray(r["outT"], dtype=np.float32) for r in res.results]
    out = np.empty((B, S, H), dtype=np.float32)
    for b in range(B):
        acc = outs[4 * b] + outs[4 * b + 1] + outs[4 * b + 2] + outs[4 * b + 3]
        out[b] = acc.T
    return out
